# revision 19
# baseline (speedup 1.0000x reference)
"""CoSent clustering loss on 8 Trainium2 NeuronCores.

Strategy: exploit S = S^T and compute only the upper triangle of the 64x64
grid of 128x128 similarity tiles (2080 tiles globally, 260 per core), at
fp8 DoubleRow matmul speed:

  * Host: sort rows by label (loss is permutation invariant), normalize in
    fp64, scale by 16 and quantize to fp8-e4m3, lay out transposed as
    eT[p, k, n] = e[n, 128k + p].  Rotate by 128*c rows per core so every
    core runs the identical program on rotated data (pure SPMD).
  * Core c owns local row-blocks r' = 8i (i = 0..7) and computes tiles
    (r', (r'+o) mod 64) for o = 1..31, plus o = 32 iff global block < 32,
    plus the diagonal tile.  Every unordered block pair is computed exactly
    once globally; each core has the same tile count (260).
  * Per row: 3 PSUM strips (12/12/9|8 blocks) via single DoubleRow fp8
    matmuls (contraction 256 in one instruction, 0.5 cyc/col).  ACT does
    exp(+s..) with a fused row-sum accumulator; the bf16 exp tiles feed
    per-tile PE ones-matmuls that produce column sums (the (j,i) ordering
    of each off-diagonal tile).  Host adds row- and col-side partials.
  * Same-label terms live only in the diagonal tile and the (r', r'+1)
    window tile (asserted from label counts).  Masks built on DVE in bf16
    select them exactly: the diagonal block is excluded from the plain
    accumulation entirely (separate exp call + (1-same) mask), so no
    large-value cancellation anywhere.
  * No collective: each core DMAs ~2.6KB of per-row/per-label partials
    out; the host does the exact O(N) segment reduction and the final log.
"""
import os
import sys

sys.path.insert(0, "/opt/trn_rl_repo")

import numpy as np
import ml_dtypes
import concourse.bacc as bacc
import concourse.bass as bass
import concourse.tile as tile
from concourse import mybir, bass_utils

F32 = mybir.dt.float32
BF16 = mybir.dt.bfloat16
F8E4 = mybir.dt.float8e4
AF = mybir.ActivationFunctionType
OP = mybir.AluOpType
DR = mybir.MatmulPerfMode.DoubleRow

N = 8192
D = 256
L = 128
NCORES = 8
NB = N // 128          # 64 column/row blocks
RPB = 8                # row-blocks per core
USE_FP8 = True

ET_DT = F8E4 if USE_FP8 else BF16
ET_NP = ml_dtypes.float8_e4m3 if USE_FP8 else ml_dtypes.bfloat16
ET_SCALE = 16.0 if USE_FP8 else 1.0  # host multiplies e by this pre-quant
# device ACT scale = s / ET_SCALE^2 (PSUM holds ET_SCALE^2 * cos)
PSUM_PER_COS = ET_SCALE * ET_SCALE


def _omax(i):
    return 32 if i < 4 else 31


def _strips(i):
    """Per-row strips: (o_start, n_blocks).  Strip 0 holds the diagonal
    block (o=0) + 11 off-diag; exp/accum skips its first 128 cols."""
    return [(0, 12), (12, 12), (24, _omax(i) - 23)]


def _runs(i, o_start, nblk):
    """Split a strip into (psum_col, local_block, n_blocks<=4) matmul runs,
    contiguous in local (rotated) block space."""
    out = []
    o = o_start
    while o < o_start + nblk:
        b = (8 * i + o) % NB
        # blocks remaining in this strip, capped by the mod-64 wrap and 4
        n = min(o_start + nblk - o, NB - b, 4)
        out.append(((o - o_start) * 128, b, n))
        o += n
    return out


_NC = None


def _build():
    nc = bacc.Bacc("TRN2", target_bir_lowering=False, debug=False,
                   num_devices=NCORES)
    # et = [idf8 128 | kidf8 128 | eT 8192] along the last axis: the fp8
    # identity pair for the diagonal-kill matmul rides the same tensor
    et_d = nc.dram_tensor("et", [128, 2, N + 256], ET_DT,
                          kind="ExternalInput")
    mylab_d = nc.dram_tensor("mylab", [128, RPB], F32, kind="ExternalInput")
    wl_d = nc.dram_tensor("wl", [RPB, 2, 128], BF16, kind="ExternalInput")
    ident_d = nc.dram_tensor("ident", [128, 128], BF16, kind="ExternalInput")
    s_d = nc.dram_tensor("s", [1, 1], F32, kind="ExternalInput")

    # packed output: 8 per-row records of
    # [btot 3 | bs_d | ad | aw | bsw | wcol 2 | bcol 32 | spare] = 42 fp32
    # (the spare holds row 0's split strip0 accum)
    RECW = 42
    out_d = nc.dram_tensor("out", [128, RPB * RECW], F32,
                           kind="ExternalOutput")

    with tile.TileContext(nc) as tc:
        with (
            tc.tile_pool(name="persist", bufs=1) as persist,
            tc.tile_pool(name="psS", bufs=2, space="PSUM") as psS,
            tc.tile_pool(name="psA", bufs=1, space="PSUM") as psA,
            tc.tile_pool(name="psB", bufs=1, space="PSUM") as psB,
            tc.tile_pool(name="strip", bufs=6) as stp,
            tc.tile_pool(name="dtile", bufs=2) as dtp,
            tc.tile_pool(name="msk", bufs=2) as mkp,
            tc.tile_pool(name="wmsk", bufs=2) as wmp,
            tc.tile_pool(name="junk", bufs=2) as jkp,
        ):
            # warm-up exp off a memset tile: ACT table load starts at t=0,
            # fully under the eT DMA
            warm_in = persist.tile([128, 1], F32)
            nc.gpsimd.memset(warm_in, 0.0)
            warm = persist.tile([128, 1], F32)
            nc.scalar.activation(warm, warm_in, AF.Exp, scale=0.0)
            ones_bf = persist.tile([128, 1], BF16)
            nc.vector.memset(ones_bf, 1.0)

            # ---- DMA order: a tiny first chunk unblocks the first matmul
            # and the diag exp; strip-aligned chunks follow; metadata rides
            # in the gap before it is needed (~4us in) ----
            eT = persist.tile([128, 2, N + 256], ET_DT)
            nc.sync.dma_start(out=eT[:, :, 0:768], in_=et_d[:, :, 0:768])
            s_bc = persist.tile([128, 1], F32)
            s_ap = s_d[0:1, 0:1]
            nc.sync.dma_start(out=s_bc, in_=bass.AP(
                tensor=s_ap.tensor, offset=s_ap.offset, ap=[[0, 128], [1, 1]]))
            for lo, hi in ((768, 1280), (1280, 1792), (1792, 3328),
                           (3328, 4608)):
                nc.sync.dma_start(out=eT[:, :, lo:hi], in_=et_d[:, :, lo:hi])
            idf8 = eT[:, :, 0:128]
            kidf8 = eT[:, :, 128:256]

            mylab = persist.tile([128, RPB], F32)
            nc.sync.dma_start(out=mylab, in_=mylab_d[:, :])
            ident = persist.tile([128, 128], BF16)
            nc.sync.dma_start(out=ident, in_=ident_d[:, :])
            wlall = persist.tile([128, 2 * RPB * 128], BF16)
            wl_ap = wl_d[0:1, 0:1, 0:1]
            nc.sync.dma_start(out=wlall, in_=bass.AP(
                tensor=wl_ap.tensor, offset=wl_ap.offset,
                ap=[[0, 128], [1, 2 * RPB * 128]]))
            for lo, hi in ((4608, 6528), (6528, 8448)):
                nc.sync.dma_start(out=eT[:, :, lo:hi], in_=et_d[:, :, lo:hi])

            # ACT scales: s/PSUM_PER_COS and its negation
            s_sc = persist.tile([128, 1], F32)
            nc.vector.tensor_scalar(out=s_sc, in0=s_bc,
                                    scalar1=1.0 / PSUM_PER_COS, scalar2=None,
                                    op0=OP.mult)

            # ---- row accumulators live inside the packed per-row records;
            # rows 0..6 ship while row 7 still computes ----
            out_sb = persist.tile([128, RPB, RECW], F32)

            # colsum slots: 32 per row (offset o-1), then 2 wcol slots/row
            bcolps = psB.tile([128, 32 * RPB + 2 * RPB], F32)
            wcolps = bcolps[:, 32 * RPB:]

            def mm_strip(i, k, name, ps0a=None):
                rb = 8 * i
                o_s, nblk = _strips(i)[k]
                ps = psS.tile([128, 1536], F32, tag="psS", name=f"ps{i}_{k}")
                for col, b, n in _runs(i, o_s, nblk):
                    diag_kill = k == 0 and col == 0
                    # row 0's first run lands in its own PSUM tile so the
                    # first exp call does not wait for the rest of strip0
                    # (tile-granular dependency tracking)
                    dst = (ps0a[:, 0:512] if ps0a is not None and col == 0
                           else ps[:, col:col + n * 128])
                    nc.tensor.matmul(
                        dst,
                        eT[:, :, 256 + rb * 128:256 + (rb + 1) * 128],
                        eT[:, :, 256 + b * 128:256 + b * 128 + n * 128],
                        perf_mode=DR, start=True, stop=not diag_kill)
                    if diag_kill:
                        nc.tensor.matmul(
                            (ps0a if ps0a is not None else ps)[:, 0:128],
                            idf8, kidf8,
                            perf_mode=DR, start=False, stop=True)
                return ps

            def colsums(i, k, es):
                o_s, nblk = _strips(i)[k]
                for o in range(max(o_s, 1), o_s + nblk):
                    sl = i * 32 + o - 1
                    nc.tensor.matmul(
                        bcolps[:, sl:sl + 1],
                        es[:, (o - o_s) * 128:(o - o_s + 1) * 128],
                        ones_bf, start=True, stop=True)

            # ---- software pipeline over the 8 row-blocks ----
            ps = {}
            ps0a = psA.tile([128, 512], F32)
            ps[(0, 0)] = mm_strip(0, 0, "s0", ps0a=ps0a)
            ps[(0, 1)] = mm_strip(0, 1, "s1")
            for i in range(RPB):
                w2 = _strips(i)[2][1] * 128

                # masks (only need label DMAs)
                msame = mkp.tile([128, 128], BF16, tag="msame")
                nc.vector.tensor_scalar(
                    out=msame, in0=wlall[:, (2 * i) * 128:(2 * i + 1) * 128],
                    scalar1=mylab[:, i:i + 1], scalar2=None, op0=OP.is_equal)
                msd = mkp.tile([128, 128], BF16, tag="msd")
                nc.vector.scalar_tensor_tensor(
                    out=msd, in0=msame, scalar=1.0, in1=ident,
                    op0=OP.mult, op1=OP.subtract)
                mw = mkp.tile([128, 128], BF16, tag="mw")
                nc.vector.tensor_scalar(
                    out=mw, in0=wlall[:, (2 * i + 1) * 128:(2 * i + 2) * 128],
                    scalar1=mylab[:, i:i + 1], scalar2=None, op0=OP.is_equal)

                # ACT: one exp call for the whole strip0 (diag killed to
                # exp(-17.5) by the matmul, excluded from a/b by masks).
                # Row 0 splits at the first matmul-run boundary so ACT can
                # start before the rest of strip0's data lands.
                es0 = stp.tile([128, 1536], BF16, tag="es", name=f"es{i}_0")
                if i == 0:
                    nc.scalar.activation(es0[:, 0:512], ps0a, AF.Exp,
                                         scale=s_sc,
                                         accum_out=out_sb[:, i, 41:42])
                    nc.scalar.activation(es0[:, 512:1536],
                                         ps[(i, 0)][:, 512:1536],
                                         AF.Exp, scale=s_sc,
                                         accum_out=out_sb[:, i, 0:1])
                else:
                    nc.scalar.activation(es0, ps[(i, 0)], AF.Exp, scale=s_sc,
                                         accum_out=out_sb[:, i, 0:1])

                # PE: strip2 matmul (psS slot of strip0 frees after main0)
                ps[(i, 2)] = mm_strip(i, 2, "s2")

                # DVE: exp(-s..) of diag+window cols via reciprocal of the
                # +s exp tile, then masked accumulations
                ef32 = dtp.tile([128, 256], F32, tag="ef32")
                nc.vector.tensor_copy(ef32, es0[:, 0:256])
                ead = dtp.tile([128, 256], F32, tag="ead")
                nc.vector.reciprocal_approx_fast(out=ead, in_=ef32)
                jb = jkp.tile([128, 128], BF16, tag="jb")
                nc.vector.scalar_tensor_tensor(
                    out=jb, in0=es0[:, 0:128], scalar=1.0, in1=msd,
                    op0=OP.mult, op1=OP.mult, accum_out=out_sb[:, i, 3:4])
                ja_d = jkp.tile([128, 128], BF16, tag="ja_d")
                nc.vector.scalar_tensor_tensor(
                    out=ja_d, in0=ead[:, 0:128], scalar=1.0, in1=msd,
                    op0=OP.mult, op1=OP.mult, accum_out=out_sb[:, i, 4:5])
                jm_w = wmp.tile([128, 128], BF16, tag="jm_w")
                nc.vector.scalar_tensor_tensor(
                    out=jm_w, in0=es0[:, 128:256], scalar=1.0, in1=mw,
                    op0=OP.mult, op1=OP.mult, accum_out=out_sb[:, i, 6:7])
                ja_w = wmp.tile([128, 128], BF16, tag="ja_w")
                nc.vector.scalar_tensor_tensor(
                    out=ja_w, in0=ead[:, 128:256], scalar=1.0, in1=mw,
                    op0=OP.mult, op1=OP.mult, accum_out=out_sb[:, i, 5:6])

                # PE: strip0 colsums + window masked colsums
                colsums(i, 0, es0)
                nc.tensor.matmul(wcolps[:, 2 * i:2 * i + 1], jm_w, ones_bf,
                                 start=True, stop=True)
                nc.tensor.matmul(wcolps[:, 2 * i + 1:2 * i + 2], ja_w, ones_bf,
                                 start=True, stop=True)

                # ACT strip1; PE next-row strip0; colsums strip1
                es1 = stp.tile([128, 1536], BF16, tag="es", name=f"es{i}_1")
                nc.scalar.activation(es1, ps[(i, 1)], AF.Exp, scale=s_sc,
                                     accum_out=out_sb[:, i, 1:2])
                if i + 1 < RPB:
                    ps[(i + 1, 0)] = mm_strip(i + 1, 0, "s0")
                colsums(i, 1, es1)

                # ACT strip2; PE next-row strip1; colsums strip2
                es2 = stp.tile([128, 1536], BF16, tag="es", name=f"es{i}_2")
                nc.scalar.activation(es2[:, 0:w2], ps[(i, 2)][:, 0:w2],
                                     AF.Exp, scale=s_sc,
                                     accum_out=out_sb[:, i, 2:3])
                if i + 1 < RPB:
                    ps[(i + 1, 1)] = mm_strip(i + 1, 1, "s1")
                colsums(i, 2, es2)
                del ps[(i, 0)], ps[(i, 1)], ps[(i, 2)]

                # stage this row's colsum partials into its output record
                nc.vector.tensor_copy(out_sb[:, i, 7:9],
                                      wcolps[:, 2 * i:2 * i + 2])
                nc.vector.tensor_copy(out_sb[:, i, 9:41],
                                      bcolps[:, i * 32:(i + 1) * 32])
                if i == RPB - 2:
                    # rows 0..6 ship while row 7 still computes
                    nc.sync.dma_start(
                        out=out_d[:, 0:(RPB - 1) * RECW],
                        in_=out_sb[:, 0:RPB - 1, :])

            # ---- last row's record ----
            nc.sync.dma_start(out=out_d[:, (RPB - 1) * RECW:RPB * RECW],
                              in_=out_sb[:, RPB - 1, :])

    nc.compile()
    return nc


def _get_nc():
    global _NC
    if _NC is None:
        _NC = _build()
    return _NC


def prepare(embeddings, labels, logit_scale):
    emb = np.asarray(embeddings, dtype=np.float64)
    lab = np.asarray(labels).astype(np.int64).reshape(-1)
    s = np.asarray(logit_scale, dtype=np.float32).reshape(1, 1)
    assert emb.shape == (N, D) and lab.shape == (N,)

    perm = np.argsort(lab, kind="stable")
    lab_s = lab[perm]
    e = emb[perm]
    e = e / np.maximum(np.linalg.norm(e, axis=1, keepdims=True), 1e-12)
    ehat = (e * ET_SCALE).astype(ET_NP)

    # same-label pairs must sit within one 128-block or span two adjacent
    # blocks (window pad = 1)
    counts = np.bincount(lab_s, minlength=L)
    starts = np.searchsorted(lab_s, np.arange(L), "left")
    ends = np.searchsorted(lab_s, np.arange(L), "right")
    nz = counts > 0
    assert (((ends[nz] - 1) // 128) - (starts[nz] // 128)).max() <= 1, \
        "a label group spans >2 blocks; window pad=1 insufficient"

    lab_bf = lab_s.astype(ml_dtypes.bfloat16)
    ident = np.eye(128, dtype=ml_dtypes.bfloat16)
    idf8 = np.ascontiguousarray(np.broadcast_to(
        np.eye(128, dtype=ET_NP), (2, 128, 128)).transpose(1, 0, 2))
    kidf8 = np.ascontiguousarray(
        -240.0 * idf8.astype(np.float32)).astype(ET_NP)
    idk = np.concatenate([idf8, kidf8], axis=2)  # [128, 2, 256]
    in_maps = []
    for c in range(NCORES):
        rot = np.roll(ehat, -128 * c, axis=0)         # [N, D]
        et = np.concatenate(
            [idk, rot.reshape(N, 2, 128).transpose(2, 1, 0)], axis=2)
        et = np.ascontiguousarray(et)
        lab_rot = np.roll(lab_bf, -128 * c)
        mylab = np.empty((128, RPB), dtype=np.float32)
        wl = np.empty((RPB, 2, 128), dtype=ml_dtypes.bfloat16)
        for i in range(RPB):
            mylab[:, i] = lab_rot[8 * i * 128:(8 * i + 1) * 128]
            wl[i, 0] = lab_rot[8 * i * 128:(8 * i + 1) * 128]
            nxt = ((8 * i + 1) % NB) * 128
            wl[i, 1] = lab_rot[nxt:nxt + 128]
        in_maps.append({
            "et": et,
            "mylab": np.ascontiguousarray(mylab),
            "wl": wl,
            "ident": ident,
            "s": s,
        })
    return in_maps, lab_s


LAST_EXEC_NS = None
LAST_RESULT = None


def kernel(embeddings, labels, logit_scale):
    in_maps, lab_s = prepare(embeddings, labels, logit_scale)
    nc = _get_nc()
    trace = bool(int(os.environ.get("KERNEL_TRACE", "0")))
    res = bass_utils.run_bass_kernel_spmd(nc, in_maps,
                                          core_ids=list(range(NCORES)),
                                          trace=trace)
    global LAST_EXEC_NS, LAST_RESULT
    LAST_EXEC_NS = res.exec_time_ns
    LAST_RESULT = res

    # ---- exact O(N) combine on host (fp64) ----
    b = np.zeros((NB, 128))
    a = np.zeros((NB, 128))
    for c in range(NCORES):
        rec = res.results[c]["out"].astype(np.float64).reshape(128, RPB, 42)
        for i in range(RPB):
            gb = (8 * i + c) % NB   # global sorted block of local block 8i
            spare = rec[:, i, 41] if i == 0 else 0.0
            b[gb] += (rec[:, i, 0:3].sum(axis=1) + spare
                      - rec[:, i, 3] - rec[:, i, 6])
            a[gb] += rec[:, i, 4] + rec[:, i, 5]
            for o in range(1, _omax(i) + 1):
                cb = (8 * i + o) % NB
                b[(cb + c) % NB] += rec[:, i, 9 + o - 1]
            wbl = ((8 * i + 1) % NB + c) % NB
            b[wbl] -= rec[:, i, 7]
            a[wbl] += rec[:, i, 8]

    b = b.reshape(-1)
    a = a.reshape(-1)
    A = np.zeros(L)
    B = np.zeros(L)
    np.add.at(A, lab_s, a)
    np.add.at(B, lab_s, b)
    counts = np.bincount(lab_s, minlength=L)
    valid = counts >= 2
    loss = np.log1p(np.sum(np.where(valid, A * B, 0.0)))
    return np.float32(loss)


# revision 21
# speedup vs baseline: 1.0109x; 1.0109x over previous
"""CoSent clustering loss on 8 Trainium2 NeuronCores.

Strategy: exploit S = S^T and compute only the upper triangle of the 64x64
grid of 128x128 similarity tiles (2080 tiles globally, 260 per core), at
fp8 DoubleRow matmul speed:

  * Host: sort rows by label (loss is permutation invariant), normalize in
    fp64, scale by 16 and quantize to fp8-e4m3, lay out transposed as
    eT[p, k, n] = e[n, 128k + p].  Rotate by 128*c rows per core so every
    core runs the identical program on rotated data (pure SPMD).
  * Core c owns local row-blocks r' = 8i (i = 0..7) and computes tiles
    (r', (r'+o) mod 64) for o = 1..31, plus o = 32 iff global block < 32,
    plus the diagonal tile.  Every unordered block pair is computed exactly
    once globally; each core has the same tile count (260).
  * Per row: 3 PSUM strips (12/12/9|8 blocks) via single DoubleRow fp8
    matmuls (contraction 256 in one instruction, 0.5 cyc/col).  ACT does
    exp(+s..) with a fused row-sum accumulator; the bf16 exp tiles feed
    per-tile PE ones-matmuls that produce column sums (the (j,i) ordering
    of each off-diagonal tile).  Host adds row- and col-side partials.
  * Same-label terms live only in the diagonal tile and the (r', r'+1)
    window tile (asserted from label counts).  A second accumulating
    matmul adds -480*I to the diagonal block so exp sends the diagonal to
    ~e-17.5; bf16 masks (same-label minus identity) then select the
    same-label sums exactly -- no large-value cancellation anywhere.  The
    exp(-s..) side comes from a DVE reciprocal of the +s exp tile.
  * No collective: each core DMAs ~2.6KB of per-row/per-label partials
    out; the host does the exact O(N) segment reduction and the final log.
"""
import os
import sys

sys.path.insert(0, "/opt/trn_rl_repo")

import numpy as np
import ml_dtypes
import concourse.bacc as bacc
import concourse.bass as bass
import concourse.tile as tile
from concourse import mybir, bass_utils

F32 = mybir.dt.float32
BF16 = mybir.dt.bfloat16
F8E4 = mybir.dt.float8e4
AF = mybir.ActivationFunctionType
OP = mybir.AluOpType
DR = mybir.MatmulPerfMode.DoubleRow

N = 8192
D = 256
L = 128
NCORES = 8
NB = N // 128          # 64 column/row blocks
RPB = 8                # row-blocks per core
USE_FP8 = True

ET_DT = F8E4 if USE_FP8 else BF16
ET_NP = ml_dtypes.float8_e4m3 if USE_FP8 else ml_dtypes.bfloat16
ET_SCALE = 16.0 if USE_FP8 else 1.0  # host multiplies e by this pre-quant
# device ACT scale = s / ET_SCALE^2 (PSUM holds ET_SCALE^2 * cos)
PSUM_PER_COS = ET_SCALE * ET_SCALE


def _omax(i):
    return 32 if i < 4 else 31


def _strips(i):
    """Per-row strips: (o_start, n_blocks).  Strip 0 holds the diagonal
    block (o=0) + 11 off-diag; exp/accum skips its first 128 cols."""
    return [(0, 12), (12, 12), (24, _omax(i) - 23)]


def _runs(i, o_start, nblk):
    """Split a strip into (psum_col, local_block, n_blocks<=4) matmul runs,
    contiguous in local (rotated) block space."""
    out = []
    o = o_start
    while o < o_start + nblk:
        b = (8 * i + o) % NB
        # blocks remaining in this strip, capped by the mod-64 wrap and 4
        n = min(o_start + nblk - o, NB - b, 4)
        out.append(((o - o_start) * 128, b, n))
        o += n
    return out


_NC = None


def _build():
    nc = bacc.Bacc("TRN2", target_bir_lowering=False, debug=False,
                   num_devices=NCORES)
    # et = [idf8 128 | kidf8 128 | eT 8192] along the last axis: the fp8
    # identity pair for the diagonal-kill matmul rides the same tensor
    et_d = nc.dram_tensor("et", [128, 2, N + 256], ET_DT,
                          kind="ExternalInput")
    mylab_d = nc.dram_tensor("mylab", [128, RPB], F32, kind="ExternalInput")
    wl_d = nc.dram_tensor("wl", [RPB, 2, 128], BF16, kind="ExternalInput")
    ident_d = nc.dram_tensor("ident", [128, 128], BF16, kind="ExternalInput")
    s_d = nc.dram_tensor("s", [1, 1], F32, kind="ExternalInput")

    # packed output: 8 per-row records of
    # [btot 3 | bs_d | ad | aw | bsw | wcol 2 | bcol 32 | spare] = 42 fp32
    # (the spare holds row 0's split strip0 accum)
    RECW = 42
    out_d = nc.dram_tensor("out", [128, RPB * RECW], F32,
                           kind="ExternalOutput")

    with tile.TileContext(nc) as tc:
        with (
            tc.tile_pool(name="persist", bufs=1) as persist,
            tc.tile_pool(name="psS", bufs=2, space="PSUM") as psS,
            tc.tile_pool(name="psA", bufs=1, space="PSUM") as psA,
            tc.tile_pool(name="psB", bufs=1, space="PSUM") as psB,
            tc.tile_pool(name="strip", bufs=6) as stp,
            tc.tile_pool(name="dtile", bufs=2) as dtp,
            tc.tile_pool(name="msk", bufs=2) as mkp,
            tc.tile_pool(name="wmsk", bufs=2) as wmp,
            tc.tile_pool(name="junk", bufs=2) as jkp,
        ):
            # warm-up exp off a memset tile: ACT table load starts at t=0,
            # fully under the eT DMA
            warm_in = persist.tile([128, 1], F32)
            nc.gpsimd.memset(warm_in, 0.0)
            warm = persist.tile([128, 1], F32)
            nc.scalar.activation(warm, warm_in, AF.Exp, scale=0.0)
            ones_bf = persist.tile([128, 1], BF16)
            nc.vector.memset(ones_bf, 1.0)

            # ---- DMA order: a tiny first chunk unblocks the first matmul
            # and the diag exp; strip-aligned chunks follow; metadata rides
            # in the gap before it is needed (~4us in) ----
            eT = persist.tile([128, 2, N + 256], ET_DT)
            nc.sync.dma_start(out=eT[:, :, 0:768], in_=et_d[:, :, 0:768])
            s_bc = persist.tile([128, 1], F32)
            s_ap = s_d[0:1, 0:1]
            nc.sync.dma_start(out=s_bc, in_=bass.AP(
                tensor=s_ap.tensor, offset=s_ap.offset, ap=[[0, 128], [1, 1]]))
            for lo, hi in ((768, 1792), (1792, 3328), (3328, 4608)):
                nc.sync.dma_start(out=eT[:, :, lo:hi], in_=et_d[:, :, lo:hi])
            idf8 = eT[:, :, 0:128]
            kidf8 = eT[:, :, 128:256]

            mylab = persist.tile([128, RPB], F32)
            nc.sync.dma_start(out=mylab, in_=mylab_d[:, :])
            ident = persist.tile([128, 128], BF16)
            nc.sync.dma_start(out=ident, in_=ident_d[:, :])
            wlall = persist.tile([128, 2 * RPB * 128], BF16)
            wl_ap = wl_d[0:1, 0:1, 0:1]
            nc.sync.dma_start(out=wlall, in_=bass.AP(
                tensor=wl_ap.tensor, offset=wl_ap.offset,
                ap=[[0, 128], [1, 2 * RPB * 128]]))
            for lo, hi in ((4608, 6528), (6528, 8448)):
                nc.sync.dma_start(out=eT[:, :, lo:hi], in_=et_d[:, :, lo:hi])

            # ACT scales: s/PSUM_PER_COS and its negation
            s_sc = persist.tile([128, 1], F32)
            nc.vector.tensor_scalar(out=s_sc, in0=s_bc,
                                    scalar1=1.0 / PSUM_PER_COS, scalar2=None,
                                    op0=OP.mult)

            # ---- row accumulators live inside the packed per-row records;
            # rows 0..6 ship while row 7 still computes ----
            out_sb = persist.tile([128, RPB, RECW], F32)

            # colsum slots: 32 per row (offset o-1), then 2 wcol slots/row
            bcolps = psB.tile([128, 32 * RPB + 2 * RPB], F32)
            wcolps = bcolps[:, 32 * RPB:]

            def mm_strip(i, k, name, ps0a=None):
                rb = 8 * i
                o_s, nblk = _strips(i)[k]
                ps = psS.tile([128, 1536], F32, tag="psS", name=f"ps{i}_{k}")
                for col, b, n in _runs(i, o_s, nblk):
                    diag_kill = k == 0 and col == 0
                    # row 0's first run lands in its own PSUM tile so the
                    # first exp call does not wait for the rest of strip0
                    # (tile-granular dependency tracking)
                    dst = (ps0a[:, 0:512] if ps0a is not None and col == 0
                           else ps[:, col:col + n * 128])
                    nc.tensor.matmul(
                        dst,
                        eT[:, :, 256 + rb * 128:256 + (rb + 1) * 128],
                        eT[:, :, 256 + b * 128:256 + b * 128 + n * 128],
                        perf_mode=DR, start=True, stop=not diag_kill)
                    if diag_kill:
                        nc.tensor.matmul(
                            (ps0a if ps0a is not None else ps)[:, 0:128],
                            idf8, kidf8,
                            perf_mode=DR, start=False, stop=True)
                return ps

            def colsums(i, k, es):
                o_s, nblk = _strips(i)[k]
                for o in range(max(o_s, 1), o_s + nblk):
                    sl = i * 32 + o - 1
                    nc.tensor.matmul(
                        bcolps[:, sl:sl + 1],
                        es[:, (o - o_s) * 128:(o - o_s + 1) * 128],
                        ones_bf, start=True, stop=True)

            # ---- software pipeline over the 8 row-blocks ----
            ps = {}
            ps0a = psA.tile([128, 512], F32)
            ps[(0, 0)] = mm_strip(0, 0, "s0", ps0a=ps0a)
            ps[(0, 1)] = mm_strip(0, 1, "s1")
            for i in range(RPB):
                w2 = _strips(i)[2][1] * 128

                # masks (only need label DMAs)
                msame = mkp.tile([128, 128], BF16, tag="msame")
                nc.vector.tensor_scalar(
                    out=msame, in0=wlall[:, (2 * i) * 128:(2 * i + 1) * 128],
                    scalar1=mylab[:, i:i + 1], scalar2=None, op0=OP.is_equal)
                msd = mkp.tile([128, 128], BF16, tag="msd")
                nc.vector.scalar_tensor_tensor(
                    out=msd, in0=msame, scalar=1.0, in1=ident,
                    op0=OP.mult, op1=OP.subtract)
                mw = mkp.tile([128, 128], BF16, tag="mw")
                nc.vector.tensor_scalar(
                    out=mw, in0=wlall[:, (2 * i + 1) * 128:(2 * i + 2) * 128],
                    scalar1=mylab[:, i:i + 1], scalar2=None, op0=OP.is_equal)

                # ACT: one exp call for the whole strip0 (diag killed to
                # exp(-17.5) by the matmul, excluded from a/b by masks).
                # Row 0 splits at the first matmul-run boundary so ACT can
                # start before the rest of strip0's data lands.
                es0 = stp.tile([128, 1536], BF16, tag="es", name=f"es{i}_0")
                if i == 0:
                    nc.scalar.activation(es0[:, 0:512], ps0a, AF.Exp,
                                         scale=s_sc,
                                         accum_out=out_sb[:, i, 41:42])
                    nc.scalar.activation(es0[:, 512:1536],
                                         ps[(i, 0)][:, 512:1536],
                                         AF.Exp, scale=s_sc,
                                         accum_out=out_sb[:, i, 0:1])
                else:
                    nc.scalar.activation(es0, ps[(i, 0)], AF.Exp, scale=s_sc,
                                         accum_out=out_sb[:, i, 0:1])

                # PE: strip2 matmul (psS slot of strip0 frees after main0)
                ps[(i, 2)] = mm_strip(i, 2, "s2")

                # DVE: exp(-s..) of diag+window cols via reciprocal of the
                # +s exp tile, then masked accumulations
                ef32 = dtp.tile([128, 256], F32, tag="ef32")
                nc.vector.tensor_copy(ef32, es0[:, 0:256])
                ead = dtp.tile([128, 256], F32, tag="ead")
                nc.vector.reciprocal_approx_fast(out=ead, in_=ef32)
                jb = jkp.tile([128, 128], BF16, tag="jb")
                nc.vector.scalar_tensor_tensor(
                    out=jb, in0=es0[:, 0:128], scalar=1.0, in1=msd,
                    op0=OP.mult, op1=OP.mult, accum_out=out_sb[:, i, 3:4])
                ja_d = jkp.tile([128, 128], BF16, tag="ja_d")
                nc.vector.scalar_tensor_tensor(
                    out=ja_d, in0=ead[:, 0:128], scalar=1.0, in1=msd,
                    op0=OP.mult, op1=OP.mult, accum_out=out_sb[:, i, 4:5])
                jm_w = wmp.tile([128, 128], BF16, tag="jm_w")
                nc.vector.scalar_tensor_tensor(
                    out=jm_w, in0=es0[:, 128:256], scalar=1.0, in1=mw,
                    op0=OP.mult, op1=OP.mult, accum_out=out_sb[:, i, 6:7])
                ja_w = wmp.tile([128, 128], BF16, tag="ja_w")
                nc.vector.scalar_tensor_tensor(
                    out=ja_w, in0=ead[:, 128:256], scalar=1.0, in1=mw,
                    op0=OP.mult, op1=OP.mult, accum_out=out_sb[:, i, 5:6])

                # PE: strip0 colsums + window masked colsums
                colsums(i, 0, es0)
                nc.tensor.matmul(wcolps[:, 2 * i:2 * i + 1], jm_w, ones_bf,
                                 start=True, stop=True)
                nc.tensor.matmul(wcolps[:, 2 * i + 1:2 * i + 2], ja_w, ones_bf,
                                 start=True, stop=True)

                # ACT strip1; PE next-row strip0; colsums strip1
                es1 = stp.tile([128, 1536], BF16, tag="es", name=f"es{i}_1")
                nc.scalar.activation(es1, ps[(i, 1)], AF.Exp, scale=s_sc,
                                     accum_out=out_sb[:, i, 1:2])
                if i + 1 < RPB:
                    ps[(i + 1, 0)] = mm_strip(i + 1, 0, "s0")
                colsums(i, 1, es1)

                # ACT strip2; PE next-row strip1; colsums strip2
                es2 = stp.tile([128, 1536], BF16, tag="es", name=f"es{i}_2")
                nc.scalar.activation(es2[:, 0:w2], ps[(i, 2)][:, 0:w2],
                                     AF.Exp, scale=s_sc,
                                     accum_out=out_sb[:, i, 2:3])
                if i + 1 < RPB:
                    ps[(i + 1, 1)] = mm_strip(i + 1, 1, "s1")
                colsums(i, 2, es2)
                del ps[(i, 0)], ps[(i, 1)], ps[(i, 2)]

                # stage this row's colsum partials into its output record
                nc.vector.tensor_copy(out_sb[:, i, 7:9],
                                      wcolps[:, 2 * i:2 * i + 2])
                nc.vector.tensor_copy(out_sb[:, i, 9:41],
                                      bcolps[:, i * 32:(i + 1) * 32])
                if i == RPB - 2:
                    # rows 0..6 ship while row 7 still computes
                    nc.sync.dma_start(
                        out=out_d[:, 0:(RPB - 1) * RECW],
                        in_=out_sb[:, 0:RPB - 1, :])

            # ---- last row's record ----
            nc.sync.dma_start(out=out_d[:, (RPB - 1) * RECW:RPB * RECW],
                              in_=out_sb[:, RPB - 1, :])

    nc.compile()
    return nc


def _get_nc():
    global _NC
    if _NC is None:
        _NC = _build()
    return _NC


def prepare(embeddings, labels, logit_scale):
    emb = np.asarray(embeddings, dtype=np.float64)
    lab = np.asarray(labels).astype(np.int64).reshape(-1)
    s = np.asarray(logit_scale, dtype=np.float32).reshape(1, 1)
    assert emb.shape == (N, D) and lab.shape == (N,)

    perm = np.argsort(lab, kind="stable")
    lab_s = lab[perm]
    e = emb[perm]
    e = e / np.maximum(np.linalg.norm(e, axis=1, keepdims=True), 1e-12)
    ehat = (e * ET_SCALE).astype(ET_NP)

    # same-label pairs must sit within one 128-block or span two adjacent
    # blocks (window pad = 1)
    counts = np.bincount(lab_s, minlength=L)
    starts = np.searchsorted(lab_s, np.arange(L), "left")
    ends = np.searchsorted(lab_s, np.arange(L), "right")
    nz = counts > 0
    assert (((ends[nz] - 1) // 128) - (starts[nz] // 128)).max() <= 1, \
        "a label group spans >2 blocks; window pad=1 insufficient"

    lab_bf = lab_s.astype(ml_dtypes.bfloat16)
    ident = np.eye(128, dtype=ml_dtypes.bfloat16)
    idf8 = np.ascontiguousarray(np.broadcast_to(
        np.eye(128, dtype=ET_NP), (2, 128, 128)).transpose(1, 0, 2))
    kidf8 = np.ascontiguousarray(
        -240.0 * idf8.astype(np.float32)).astype(ET_NP)
    idk = np.concatenate([idf8, kidf8], axis=2)  # [128, 2, 256]
    in_maps = []
    for c in range(NCORES):
        rot = np.roll(ehat, -128 * c, axis=0)         # [N, D]
        et = np.concatenate(
            [idk, rot.reshape(N, 2, 128).transpose(2, 1, 0)], axis=2)
        et = np.ascontiguousarray(et)
        lab_rot = np.roll(lab_bf, -128 * c)
        mylab = np.empty((128, RPB), dtype=np.float32)
        wl = np.empty((RPB, 2, 128), dtype=ml_dtypes.bfloat16)
        for i in range(RPB):
            mylab[:, i] = lab_rot[8 * i * 128:(8 * i + 1) * 128]
            wl[i, 0] = lab_rot[8 * i * 128:(8 * i + 1) * 128]
            nxt = ((8 * i + 1) % NB) * 128
            wl[i, 1] = lab_rot[nxt:nxt + 128]
        in_maps.append({
            "et": et,
            "mylab": np.ascontiguousarray(mylab),
            "wl": wl,
            "ident": ident,
            "s": s,
        })
    return in_maps, lab_s


LAST_EXEC_NS = None
LAST_RESULT = None


def kernel(embeddings, labels, logit_scale):
    in_maps, lab_s = prepare(embeddings, labels, logit_scale)
    nc = _get_nc()
    trace = bool(int(os.environ.get("KERNEL_TRACE", "0")))
    res = bass_utils.run_bass_kernel_spmd(nc, in_maps,
                                          core_ids=list(range(NCORES)),
                                          trace=trace)
    global LAST_EXEC_NS, LAST_RESULT
    LAST_EXEC_NS = res.exec_time_ns
    LAST_RESULT = res

    # ---- exact O(N) combine on host (fp64) ----
    b = np.zeros((NB, 128))
    a = np.zeros((NB, 128))
    for c in range(NCORES):
        rec = res.results[c]["out"].astype(np.float64).reshape(128, RPB, 42)
        for i in range(RPB):
            gb = (8 * i + c) % NB   # global sorted block of local block 8i
            spare = rec[:, i, 41] if i == 0 else 0.0
            b[gb] += (rec[:, i, 0:3].sum(axis=1) + spare
                      - rec[:, i, 3] - rec[:, i, 6])
            a[gb] += rec[:, i, 4] + rec[:, i, 5]
            for o in range(1, _omax(i) + 1):
                cb = (8 * i + o) % NB
                b[(cb + c) % NB] += rec[:, i, 9 + o - 1]
            wbl = ((8 * i + 1) % NB + c) % NB
            b[wbl] -= rec[:, i, 7]
            a[wbl] += rec[:, i, 8]

    b = b.reshape(-1)
    a = a.reshape(-1)
    A = np.zeros(L)
    B = np.zeros(L)
    np.add.at(A, lab_s, a)
    np.add.at(B, lab_s, b)
    counts = np.bincount(lab_s, minlength=L)
    valid = counts >= 2
    loss = np.log1p(np.sum(np.where(valid, A * B, 0.0)))
    return np.array(loss, dtype=np.float32)
